# revision 51
# baseline (speedup 1.0000x reference)
"""GQA attention (B=1, L=2048, D=2048, 32 q heads, 8 kv heads, hd=64) with RoPE,
causal mask, and output projection, on 8 Trainium2 NeuronCores.

Sharding: tensor-parallel over heads. Core c owns kv head c and q heads
4c..4c+3. Each core computes its heads' attention and a partial output
projection y_c = attn_out_c @ Wo[:, 256c:256c+256].T; the host sums the 8
partials.

Design (all PE operands f16, fp32 psum accumulation):
  - Q^T: two psum tiles (RoPE tops | bottoms of 4 packed heads); rope runs
    on f16 SBUF copies and the combine writes partition-rebased 32-row
    slices straight into per-head [top;bot] qh tiles; a gpsimd SBUF DMA
    duplicates each head's 64 rows to partitions 64:127.
  - Scores: per key-tile PAIR, two K=64 matmuls packed into disjoint PE
    row groups via tile_position (even ki at rows 0:63, odd at 64:127,
    reading the duplicated qh rows) -- they run concurrently, ~2x the
    K=64 rate.  K^T is stored pair-interleaved [128, NKT/2, 128].
  - Scores psum is a flat [128, 1024] bank pair; ONE exp per pair (the
    mid dead-zone of diagonal pairs is exp'd but never read), halving
    scalar-engine call overhead.
  - K/V natural layout via PE transposes into bitcast f16 slices of the
    freed projection psum banks (alternating two banks).
  - Causal mask = multiplicative 0/1 triangle on exp(S^T) in f16 SBUF.
  - Softmax denominator (ones column of the PV stationary): DVE copy to
    partition 0, reciprocal_approx_fast, gpsimd partition_broadcast,
    DVE multiply (custom DVE ops cannot read rebased partitions).
  - A fine-grained filler queue threads projection-group and Wo-row-tile
    matmuls between attention ki-pairs so the PE never idles long enough
    for the HAM clock gate to re-throttle it, and a ~150-matmul warm-up
    burst spans the initial DMA wait (also pins run-to-run variance to
    well under 1%).
  - PSUM budget (8 banks): proj qa/qb/kv (3, reused by Wo psum and the
    transposes), score pairs (2x2), PV (1).
"""

import numpy as np
from collections import deque

L = 2048
D = 2048
HD = 64
N_HEADS = 32
N_KV = 8
NCORES = 8
QH = N_HEADS // N_KV  # q heads per core = 4
ROPE_THETA = 10000.0

LG = 512  # projection group width
AG = 512  # attention query-group width
NG = L // LG  # 4
NAG = L // AG  # 4
NKT = L // 128  # 16 key tiles
NDT = D // 128  # 16 contraction tiles

_CACHE = {}


def _build_program(dbg=False):
    import concourse.tile as tile
    import concourse.mybir as mybir
    from concourse import bacc

    f32 = mybir.dt.float32
    f16 = mybir.dt.float16
    Exp = mybir.ActivationFunctionType.Exp

    nc = bacc.Bacc("TRN2", target_bir_lowering=False, debug=False,
                   num_devices=NCORES)
    if dbg:
        d_qh = nc.dram_tensor("d_qh", [64, QH, L], f16, kind="ExternalOutput")
        d_kvn = nc.dram_tensor("d_kvn", [128, NKT, 144], f16,
                               kind="ExternalOutput")
        d_kt = nc.dram_tensor("d_kt", [128, NKT // 2, 128], f16,
                              kind="ExternalOutput")
        d_ao = nc.dram_tensor("d_ao", [128, 2, L], f16, kind="ExternalOutput")

    xT = nc.dram_tensor("xT", [128, NDT, L], f16, kind="ExternalInput")
    wq = nc.dram_tensor("wq", [128, NDT, 256], f16, kind="ExternalInput")
    wkv = nc.dram_tensor("wkv", [128, NDT, 128], f16, kind="ExternalInput")
    wo = nc.dram_tensor("wo", [128, 2, D], f16, kind="ExternalInput")
    cs4 = nc.dram_tensor("cs4", [128, 2, L], f16, kind="ExternalInput")
    csT = nc.dram_tensor("csT", [128, 2, NKT, 32], f16, kind="ExternalInput")
    misc = nc.dram_tensor("misc", [128, 272], f16, kind="ExternalInput")
    y = nc.dram_tensor("y", [L, D], f16, kind="ExternalOutput")

    with tile.TileContext(nc) as tc:
        with (
            tc.tile_pool(name="consts", bufs=1) as consts,
            tc.tile_pool(name="persist", bufs=1) as persist,
            tc.tile_pool(name="xin", bufs=12) as xin,
            tc.tile_pool(name="rope", bufs=3) as ropep,
            tc.tile_pool(name="pt", bufs=6) as ptp,
            tc.tile_pool(name="small", bufs=3) as smallp,
            tc.tile_pool(name="ysb", bufs=4) as ysbp,
            tc.tile_pool(name="mmps", bufs=1, space="PSUM") as mmps,
            tc.tile_pool(name="stps", bufs=2, space="PSUM") as stps,
            tc.tile_pool(name="pvps", bufs=1, space="PSUM") as pvps,
        ):
            # weights first: they gate the first matmuls
            wq_sb = consts.tile([128, NDT, 256], f16)
            for q in range(4):
                nc.scalar.dma_start(wq_sb[:, 4 * q:4 * q + 4, :],
                                    wq.ap()[:, 4 * q:4 * q + 4, :])
            wkv_sb = consts.tile([128, NDT, 128], f16)
            nc.gpsimd.dma_start(wkv_sb[:], wkv.ap())

            qh_sb = [persist.tile([128, L], f16, tag=f"qh{h}", name=f"qh{h}")
                     for h in range(QH)]
            kvnat = persist.tile([128, NKT, 144], f16, tag="kvnat")
            krot = persist.tile([128, NKT, 64], f16, tag="krot")
            kT2 = persist.tile([128, NKT // 2, 128], f16, tag="kT")
            ao = [persist.tile([128, L], f16, tag=f"ao{t}", name=f"ao{t}")
                  for t in range(2)]

            proj_ps = {}
            rope_done = set()

            def proj_oi(g, ob, oi):
                gsl = slice(g * LG, (g + 1) * LG)
                if ob == 0 and oi == 0:
                    proj_ps[g] = (
                        mmps.tile([128, LG], f32, tag="qa", name=f"qa{g}"),
                        mmps.tile([128, LG], f32, tag="qb", name=f"qb{g}"),
                        mmps.tile([128, LG], f32, tag="kv", name=f"kv{g}"),
                    )
                ps_qa, ps_qb, ps_kv = proj_ps[g]
                xt = proj_ps[(g, ob)]
                o = 4 * ob + oi
                st, sp = (o == 0), (o == NDT - 1)
                nc.tensor.matmul(ps_qa[:], wq_sb[:, o, 0:128],
                                 xt[:, oi, :], start=st, stop=sp)
                nc.tensor.matmul(ps_qb[:], wq_sb[:, o, 128:256],
                                 xt[:, oi, :], start=st, stop=sp)
                nc.tensor.matmul(ps_kv[:], wkv_sb[:, o, :],
                                 xt[:, oi, :], start=st, stop=sp)

            def rope_q(g):
                gsl = slice(g * LG, (g + 1) * LG)
                ps_qa, ps_qb, _ = proj_ps[g]
                qa16 = ropep.tile([128, LG], f16, tag="qa16")
                nc.scalar.copy(qa16[:], ps_qa[:])
                qb16 = ropep.tile([128, LG], f16, tag="qb16")
                nc.scalar.copy(qb16[:], ps_qb[:])
                t_a = ropep.tile([128, LG], f16, tag="t_a")
                nc.vector.tensor_mul(out=t_a[:], in0=qa16[:], in1=cs4_sb[:, 0, gsl])
                t_b = ropep.tile([128, LG], f16, tag="t_b")
                nc.vector.tensor_mul(out=t_b[:], in0=qb16[:], in1=cs4_sb[:, 1, gsl])
                t_c = ropep.tile([128, LG], f16, tag="t_c")
                nc.vector.tensor_mul(out=t_c[:], in0=qa16[:], in1=cs4_sb[:, 1, gsl])
                t_d = ropep.tile([128, LG], f16, tag="t_d")
                nc.vector.tensor_mul(out=t_d[:], in0=qb16[:], in1=cs4_sb[:, 0, gsl])
                for j in range(QH):
                    ssl = slice(32 * j, 32 * j + 32)
                    nc.vector.tensor_sub(out=qh_sb[j][0:32, gsl],
                                         in0=t_a[ssl, :], in1=t_b[ssl, :])
                    nc.vector.tensor_add(out=qh_sb[j][32:64, gsl],
                                         in0=t_c[ssl, :], in1=t_d[ssl, :])
                    # duplicate rows 0:64 -> 64:128 for the odd-ki
                    # row-group-64 score tiles (DMA keeps DVE free)
                    nc.gpsimd.dma_start(qh_sb[j][64:128, gsl],
                                        qh_sb[j][0:64, gsl])

            def rope_kv(g):
                _, _, ps_kv = proj_ps.pop(g)
                for ob in range(4):
                    proj_ps.pop((g, ob), None)
                kv16 = ropep.tile([128, LG], f16, tag="kv16")
                nc.vector.tensor_copy(out=kv16[:], in_=ps_kv[:])
                # PE transposes into bitcast slices of the freed proj psum
                # banks; alternate two banks so copy-out overlaps transpose
                tpa = mmps.tile([128, LG], f32, tag="kv", name=f"tpa{g}")
                tpb = mmps.tile([128, LG], f32, tag="qa", name=f"tpb{g}")
                for t in range(4):
                    ki = 4 * g + t
                    dst = (tpa, tpb)[t % 2][:, 64 * (t // 2):64 * (t // 2) + 64]
                    dst = dst.bitcast(f16)
                    nc.tensor.transpose(dst, kv16[:, 128 * t:128 * t + 128],
                                        eye_sb[:])
                    nc.vector.tensor_copy(out=kvnat[:, ki, 0:128], in_=dst)
                ksl = slice(4 * g, 4 * g + 4)
                u1 = ropep.tile([128, 4, 32], f16, tag="u1")
                nc.vector.tensor_mul(out=u1[:], in0=kvnat[:, ksl, 0:32],
                                     in1=csT_sb[:, 0, ksl, :])
                u2 = ropep.tile([128, 4, 32], f16, tag="u2")
                nc.vector.tensor_mul(out=u2[:], in0=kvnat[:, ksl, 32:64],
                                     in1=csT_sb[:, 1, ksl, :])
                nc.vector.tensor_sub(out=krot[:, ksl, 0:32], in0=u1[:], in1=u2[:])
                u3 = ropep.tile([128, 4, 32], f16, tag="u1")
                nc.vector.tensor_mul(out=u3[:], in0=kvnat[:, ksl, 0:32],
                                     in1=csT_sb[:, 1, ksl, :])
                u4 = ropep.tile([128, 4, 32], f16, tag="u2")
                nc.vector.tensor_mul(out=u4[:], in0=kvnat[:, ksl, 32:64],
                                     in1=csT_sb[:, 0, ksl, :])
                nc.vector.tensor_add(out=krot[:, ksl, 32:64], in0=u3[:], in1=u4[:])
                for t in range(4):
                    ki = 4 * g + t
                    dst = (tpa, tpb)[t % 2][0:64, 128 + 64 * (t // 2):
                                            128 + 64 * (t // 2) + 64]
                    dst = dst.bitcast(f16)
                    nc.tensor.transpose(dst, krot[:, ki, :], eye_sb[:])
                    rb_ = 64 * (ki % 2)
                    nc.vector.tensor_copy(out=kT2[rb_:rb_ + 64, ki // 2, :],
                                          in_=dst)
                rope_done.add(g)

            def wo_part(j, m, gn):
                gm = 4 * j + m
                msl = slice(gm * 128, gm * 128 + 128)
                if gn == 0:
                    proj_ps[("ys", j, m)] = ysbp.tile([128, D], f16, tag="ys",
                                                      name=f"ys{j}_{m}")
                ys = proj_ps[("ys", j, m)]
                nsl = slice(gn * LG, (gn + 1) * LG)
                yp = mmps.tile([128, LG], f32, tag=("qa", "qb", "kv")[gn % 3],
                               name=f"yp{j}_{m}_{gn}")
                nc.tensor.matmul(yp[:], ao[0][:, msl], wo_sb[:, 0, nsl],
                                 start=True, stop=False)
                nc.tensor.matmul(yp[:], ao[1][:, msl], wo_sb[:, 1, nsl],
                                 start=False, stop=True)
                if gn == 3 and j == 0:
                    nc.scalar.copy(ys[:, nsl], yp[:])
                else:
                    nc.vector.tensor_copy(out=ys[:, nsl], in_=yp[:])

            def wo_dma(j, m):
                gm = 4 * j + m
                msl = slice(gm * 128, gm * 128 + 128)
                ys = proj_ps.pop(("ys", j, m))
                eng = nc.sync if j == 3 else nc.gpsimd
                eng.dma_start(y.ap()[msl, :], ys[:])

            fq = deque()

            def pop_fill(n=1):
                for _ in range(n):
                    if not fq:
                        return
                    fq.popleft()()

            def attn_head(j, h):
                base = j * AG
                jsl = slice(base, base + AG)
                npair = 2 * j + 2
                pvt = pvps.tile([65, AG], f32, tag="pv", name=f"pv{j}_{h}")
                for kp in range(npair):
                    stp = stps.tile([128, 2 * AG], f32, tag="st",
                                    name=f"st{j}_{h}_{kp}")
                    pt = ptp.tile([128, 2 * AG], f16, tag="pt",
                                  name=f"pt{j}_{h}_{kp}")
                    offs = []
                    for t in range(2):
                        ki = 2 * kp + t
                        off = max(0, 128 * ki - base)
                        offs.append(off)
                        rb_ = 64 * t
                        nc.tensor.matmul(
                            stp[:, t * AG + off:(t + 1) * AG],
                            kT2[rb_:rb_ + 64, kp, :],
                            qh_sb[h][rb_:rb_ + 64, base + off:base + AG],
                            start=True, stop=True, tile_position=(rb_, 0))
                    nc.scalar.activation(pt[:, offs[0]:2 * AG],
                                         stp[:, offs[0]:2 * AG], Exp)
                    for t in range(2):
                        ki = 2 * kp + t
                        off = offs[t]
                        lo = t * AG + off
                        if ki >= 4 * j:
                            # diagonal: 0/1 triangle on the first live 128 cols
                            nc.vector.tensor_mul(out=pt[:, lo:lo + 128],
                                                 in0=pt[:, lo:lo + 128],
                                                 in1=tri_sb[:])
                            nc.tensor.matmul(pvt[:, off:off + 128],
                                             kvnat[:, ki, 64:129],
                                             pt[:, lo:lo + 128],
                                             start=(ki == 0), stop=True)
                            if off + 128 < AG:
                                nc.tensor.matmul(pvt[:, off + 128:AG],
                                                 kvnat[:, ki, 64:129],
                                                 pt[:, lo + 128:(t + 1) * AG],
                                                 start=(ki == 0), stop=False)
                        else:
                            nc.tensor.matmul(pvt[:, :], kvnat[:, ki, 64:129],
                                             pt[:, t * AG:(t + 1) * AG],
                                             start=(ki == 0), stop=False)
                    pop_fill(1)
                # normalize via the ones-row denominator
                r0 = smallp.tile([1, AG], f32, tag="r0", name=f"r0_{j}_{h}")
                nc.vector.tensor_copy(out=r0[:], in_=pvt[64:65, :])
                r1 = smallp.tile([1, AG], f32, tag="r1", name=f"r1_{j}_{h}")
                nc.vector.reciprocal_approx_fast(out=r1[:], in_=r0[:])
                rb = smallp.tile([64, AG], f32, tag="rb", name=f"rb{j}_{h}")
                nc.gpsimd.partition_broadcast(rb[:], r1[:])
                nc.vector.tensor_mul(
                    out=ao[h // 2][64 * (h % 2):64 * (h % 2) + 64, jsl],
                    in0=pvt[0:64, :], in1=rb[:])
                pop_fill(3)

            xt_issued = set()

            def issue_xt(g):
                if g in xt_issued:
                    return
                xt_issued.add(g)
                gsl = slice(g * LG, (g + 1) * LG)
                for ob in range(4):
                    xt = xin.tile([128, 4, LG], f16, tag="xt",
                                  name=f"xt{g}_{ob}")
                    nc.sync.dma_start(xt[:], xT.ap()[:, 4 * ob:4 * ob + 4, gsl])
                    proj_ps[(g, ob)] = xt

            # ---- HAM warm-up: dummy matmuls spanning the initial DMA
            # wait so the first real matmuls run at 2.4 GHz (results unused)
            warm_in = consts.tile([128, 64], f16)
            nc.vector.memset(warm_in[:], 0.0)
            warm_ps = stps.tile([128, 2 * AG], f32, tag="st", name="warm")
            for w in range(190):
                nc.tensor.matmul(warm_ps[0:64, 0:64], warm_in[:], warm_in[:],
                                 start=True, stop=True)

            # ---- startup: group-0 projections ----
            issue_xt(0)
            issue_xt(1)
            for ob in range(4):
                for oi in range(4):
                    proj_oi(0, ob, oi)

            cs4_sb = consts.tile([128, 2, L], f16)
            nc.gpsimd.dma_start(cs4_sb[:], cs4.ap())
            csT_sb = consts.tile([128, 2, NKT, 32], f16)
            nc.gpsimd.dma_start(csT_sb[:], csT.ap())
            misc_sb = consts.tile([128, 272], f16)
            nc.gpsimd.dma_start(misc_sb[:], misc.ap())
            tri_sb = misc_sb[:, 0:128]
            eye_sb = misc_sb[:, 144:272]
            nc.gpsimd.dma_start(kvnat[:, :, 128], misc.ap()[:, 128:144])
            wo_sb = consts.tile([128, 2, D], f16)

            rope_q(0)
            rope_kv(0)

            def push_proj(g):
                issue_xt(g)
                for ob in range(4):
                    for oi in range(4):
                        fq.append(lambda g=g, ob=ob, oi=oi: proj_oi(g, ob, oi))
                fq.append(lambda g=g: rope_q(g))
                fq.append(lambda g=g: rope_kv(g))

            def push_wo(j):
                for m in range(4):
                    for gn in range(4):
                        fq.append(lambda j=j, m=m, gn=gn: wo_part(j, m, gn))
                    fq.append(lambda j=j, m=m: wo_dma(j, m))

            for j in range(NAG):
                # everything queued two groups back must be emitted before
                # this group's attention consumes its outputs
                if j == 0:
                    push_proj(1)
                    push_proj(2)
                elif j + 2 <= NAG - 1:
                    push_proj(j + 2)
                if j == 1:
                    # wo weights are first needed here; deferring the 1MB DMA
                    # keeps startup bandwidth for the critical x/Wq transfers
                    nc.gpsimd.dma_start(wo_sb[:], wo.ap())
                if j >= 1:
                    push_wo(j - 1)
                for h in range(QH):
                    attn_head(j, h)
                # hard deadline: group j+1 inputs must exist before attn j+1
                while j + 1 < NAG and (j + 1) not in rope_done:
                    fq.popleft()()
            while fq:
                fq.popleft()()
            for m in range(4):
                for gn in range(4):
                    wo_part(3, m, gn)
                wo_dma(3, m)

            if dbg:
                for h in range(QH):
                    nc.sync.dma_start(d_qh.ap()[:, h, :], qh_sb[h][0:64, :])
                nc.sync.dma_start(d_kvn.ap(), kvnat[:])
                nc.sync.dma_start(d_kt.ap(), kT2[:])
                for t in range(2):
                    nc.sync.dma_start(d_ao.ap()[:, t, :], ao[t][:])

    nc.compile()
    return nc


def _host_prep(x, attn_scale, Wq, Wk, Wv, Wo):
    """Build the 8 per-core input maps."""
    xT = np.ascontiguousarray(x.reshape(L, D).T)  # [D, L]
    xT_dev = np.ascontiguousarray(
        xT.reshape(NDT, 128, L).transpose(1, 0, 2)).astype(np.float16)

    pos = np.arange(L, dtype=np.float64)
    inv_freq = 1.0 / (ROPE_THETA ** (np.arange(0, HD, 2, dtype=np.float64) / HD))
    ang = pos[:, None] * inv_freq[None, :]  # [L, 32]
    cos = np.cos(ang).astype(np.float32)  # [L, 32]
    sin = np.sin(ang).astype(np.float32)
    cs4 = np.stack([np.tile(cos.T, (4, 1)), np.tile(sin.T, (4, 1))],
                   axis=1).astype(np.float16)  # [128, 2, L]
    csT = np.stack([cos.reshape(NKT, 128, 32).transpose(1, 0, 2),
                    sin.reshape(NKT, 128, 32).transpose(1, 0, 2)],
                   axis=1).astype(np.float16)  # [128, 2, NKT, 32]

    p = np.arange(128)
    tri = (p[:, None] <= p[None, :]).astype(np.float16)
    misc = np.concatenate([tri, np.ones((128, 16), np.float16),
                           np.eye(128, dtype=np.float16)],
                          axis=1).astype(np.float16)  # [128, 272]

    kscale = float(attn_scale.reshape(-1)[0]) * HD ** -0.5

    in_maps = []
    for c in range(NCORES):
        rows_a = [Wq[256 * c + 64 * j:256 * c + 64 * j + 32] for j in range(QH)]
        rows_b = [Wq[256 * c + 64 * j + 32:256 * c + 64 * j + 64]
                  for j in range(QH)]
        WqAB = np.concatenate(rows_a + rows_b, axis=0)  # [256, D]
        wq_dev = np.ascontiguousarray(
            WqAB.T.reshape(NDT, 128, 256).transpose(1, 0, 2)).astype(np.float16)

        Wk_c = Wk[64 * c:64 * c + 64] * kscale
        Wv_c = Wv[64 * c:64 * c + 64]
        WKV = np.concatenate([Wk_c, Wv_c], axis=0)  # [128, D]
        wkv_dev = np.ascontiguousarray(
            WKV.T.reshape(NDT, 128, 128).transpose(1, 0, 2)).astype(np.float16)

        WoT_c = Wo[:, 256 * c:256 * c + 256].T  # [256, D]
        wo_dev = np.ascontiguousarray(
            WoT_c.reshape(2, 128, D).transpose(1, 0, 2)).astype(np.float16)

        in_maps.append({
            "xT": xT_dev, "wq": wq_dev, "wkv": wkv_dev, "wo": wo_dev,
            "cs4": cs4, "csT": csT, "misc": misc,
        })
    return in_maps


def _get_program(dbg=False):
    key = f"nc{int(dbg)}"
    if key not in _CACHE:
        _CACHE[key] = _build_program(dbg)
    return _CACHE[key]


def run(inputs, trace=False, dbg=False):
    """Run on 8 NeuronCores; returns (y_full, BassKernelResults)."""
    from concourse import bass_utils

    in_maps = _host_prep(inputs["x"], inputs["attn_scale"], inputs["Wq"],
                         inputs["Wk"], inputs["Wv"], inputs["Wo"])
    nc = _get_program(dbg)
    res = bass_utils.run_bass_kernel_spmd(
        nc, in_maps, core_ids=list(range(NCORES)), trace=trace)
    parts = np.stack([res.results[c]["y"] for c in range(NCORES)])
    y = parts.sum(axis=0, dtype=np.float64).astype(np.float32)
    return y.reshape(1, L, D), res


def kernel(**inputs):
    y, _ = run(inputs, trace=False)
    return y
